# revision 52
# baseline (speedup 1.0000x reference)
"""Trainium2 Bass kernel for a 2-layer relational GraphSAGE VGAE encoder.

Contract: kernel(**inputs) takes the FULL unsharded inputs (as produced by
setup_inputs()) and returns the full (mu, logvar) tuple.

Strategy (8 NeuronCores, SPMD single NEFF):
  - Nodes block-sharded: core c owns nodes [c*2500, (c+1)*2500), relabeled
    within the core by descending out-degree and padded to 2560 positions
    (5 groups of 512).
  - Segment-mean is a sequence of (gather 128 src rows) @ (one-hot 1/cnt)
    matmuls. Edges of a cell are bucketed into disjoint destination-column
    windows, each window small enough that every core has <=128 edges in it;
    one narrow matmul per (window, feature-chunk) writes its own psum slice
    with start=stop=True, so the whole cell costs only ~512 output columns
    per feature chunk instead of 512 per 128-edge chunk.
  - Layer-1 dense and the layer-2 projections are fp16 matmuls with fp32
    PSUM. BatchNorm (eval) is folded into the layer-2 weights on the host.
    The layer-2 projection is computed node-major (lhsT = h chunks, rhs =
    stacked [tab|self] weight block, 1024 wide), which both halves the
    LdWeights count and directly emits rows for the AllGather - no PE
    transposes anywhere.
  - The projected table rows are AllGather'd per node group (fp16) into 5
    shared tables (one per source group). Layer-2 aggregation is split by
    source group; the pass for source group s is interleaved into the
    layer-1 work of group s+2, so its gathers overlap compute and partial
    sums accumulate in an SBUF fp16 accumulator. Group 4 holds the
    lowest-out-degree nodes, keeping the post-AllGather tail short.
"""
import sys

sys.path.insert(0, "/opt/trn_rl_repo")

import numpy as np

NCORES = 8
N = 20000
E = 100000
IN = 512
HID = 512
CAT = 2560
OUT = 256
BN_EPS = 1e-5

NLOC = N // NCORES          # 2500
NPAD = 2560                 # 5 * 512
NG = NPAD // 512            # 5 node groups of 512 per core
NREL = 5
P = 128


# ----------------------------------------------------------------------------
# Host-side preprocessing: relabeling, window chunking, weight folding
# ----------------------------------------------------------------------------

def _windows(counts, force):
    """Split [0,512) into consecutive windows such that every core has
    <=128 edges per window.  counts: [NCORES, 512] per-core per-col edge
    counts.  force: cols where a boundary is mandatory."""
    wins = []
    lo = 0
    run = np.zeros(NCORES, np.int64)
    for col in range(512):
        c = counts[:, col]
        assert (c <= P).all(), "single column exceeds 128 edges on a core"
        if col > lo and ((run + c > P).any() or col in force):
            wins.append((lo, col))
            lo = col
            run = np.zeros(NCORES, np.int64)
        run += c
    wins.append((lo, 512))
    return wins


def _pack_cell(e, dcore, dcol, idxval, aval, wins):
    """Pack a cell's edges into per-window chunks.

    Returns idxs [nwin, NCORES, P] int32 and av [NCORES, P, 512] f16."""
    nwin = len(wins)
    los = np.array([w[0] for w in wins])
    av = np.zeros((NCORES, P, 512), np.float16)
    idxs = np.zeros((nwin, NCORES, P), np.int32)
    if len(e) == 0:
        return idxs, av
    wi = np.searchsorted(los, dcol[e], side="right") - 1
    key = dcore[e] * nwin + wi
    order = np.argsort(key, kind="stable")
    ke = key[order]
    first = np.r_[True, ke[1:] != ke[:-1]]
    runstart = np.flatnonzero(first)
    rid = np.cumsum(first) - 1
    rowp = np.arange(len(ke)) - runstart[rid]
    assert (rowp < P).all()
    eo = e[order]
    cc = ke // nwin
    ww = ke % nwin
    idxs[ww, cc, rowp] = idxval[eo]
    av[cc, rowp, dcol[eo]] = aval[eo]
    return idxs, av


def _preprocess(x, edge_index, edge_attr, Wl5, Wr5, bl5,
                Wmu_l, Wmu_r, bmu, Wlv_l, Wlv_r, blv,
                gamma, beta, run_mean, run_var):
    x = np.asarray(x, np.float32)
    src = np.asarray(edge_index[0], np.int64)
    dst = np.asarray(edge_index[1], np.int64)
    rel = np.asarray(edge_attr, np.int64)

    # --- relabel nodes within each core by descending out-degree ---
    outdeg = np.bincount(src, minlength=N)
    pos = np.empty(N, np.int64)
    for c in range(NCORES):
        ids = np.arange(c * NLOC, (c + 1) * NLOC)
        order = ids[np.argsort(-outdeg[ids], kind="stable")]
        pos[order] = np.arange(NLOC)

    cnt1 = np.bincount(rel * N + dst, minlength=NREL * N).reshape(NREL, N)
    inv1 = (1.0 / np.maximum(cnt1, 1.0)).astype(np.float32)
    cnt2 = np.bincount(dst, minlength=N)
    inv2 = (1.0 / np.maximum(cnt2, 1.0)).astype(np.float32)

    dcore = dst // NLOC
    dpos = pos[dst]
    dgrp = dpos // 512
    dcol = dpos % 512
    spos = pos[src]
    sgrp = spos // 512
    srow = ((src // NLOC) * 512 + spos % 512).astype(np.int32)

    # --- L1 cells: (group, rel) in program order ---
    win1 = []
    i1 = []
    v1 = []
    v1e = inv1[rel, dst].astype(np.float32)
    for g in range(NG):
        for k in range(NREL):
            e = np.flatnonzero((dgrp == g) & (rel == k))
            cnt = np.zeros((NCORES, 512), np.int64)
            np.add.at(cnt, (dcore[e], dcol[e]), 1)
            wins = _windows(cnt, ())
            win1.append(tuple(wins))
            idxs, av = _pack_cell(e, dcore, dcol, src.astype(np.int32),
                                  v1e, wins)
            i1.append(idxs)
            v1.append(av)
    C1 = sum(len(w) for w in win1)
    a1i = np.concatenate(i1, axis=0).transpose(1, 2, 0)          # [NC, P, C1]
    a1v = np.stack(v1, axis=0).transpose(1, 2, 0, 3).reshape(
        NCORES, P, len(v1) * 512)                                # [NC, P, 25*512]

    # --- L2 cells: (src-group, dst-group) in program order ---
    win2 = []
    i2 = []
    v2 = []
    v2e = inv2[dst].astype(np.float32)
    for s in range(NG):
        for d in range(NG):
            e = np.flatnonzero((dgrp == d) & (sgrp == s))
            cnt = np.zeros((NCORES, 512), np.int64)
            np.add.at(cnt, (dcore[e], dcol[e]), 1)
            wins = _windows(cnt, ())
            win2.append(tuple(wins))
            idxs, av = _pack_cell(e, dcore, dcol, srow, v2e, wins)
            i2.append(idxs)
            v2.append(av)
    C2 = sum(len(w) for w in win2)
    a2i = np.concatenate(i2, axis=0).transpose(1, 2, 0)          # [NC, P, C2]
    a2v = np.stack(v2, axis=0).transpose(1, 2, 0, 3).reshape(
        NCORES, P, len(v2) * 512)

    # --- node features ---
    xtab = x.astype(np.float16)                                  # [N, 512]
    xt = np.zeros((NCORES, P, NG * 2048), np.float16)
    for c in range(NCORES):
        ids = np.arange(c * NLOC, (c + 1) * NLOC)
        arr = np.zeros((NPAD, IN), np.float32)
        arr[pos[ids]] = x[ids]
        # xt[c][p, g*2048 + kc*512 + col] = arr[g*512 + col, kc*128 + p]
        xt[c] = np.ascontiguousarray(
            arr.T.reshape(4, P, NG, 512).transpose(1, 2, 0, 3)
            .reshape(P, NG * 2048)).astype(np.float16)

    # --- weight folding (BN eval folded into layer-2 weights) ---
    f64 = np.float64
    s = np.asarray(gamma, f64) / np.sqrt(np.asarray(run_var, f64) + BN_EPS)
    t = np.asarray(beta, f64) - np.asarray(run_mean, f64) * s

    # wl/wr: [P, k*2048 + kc*512 + j] = Wx[k].T[kc*128+p, j]
    def _pmaj_w(W5):
        wt = np.asarray(W5, np.float32).transpose(0, 2, 1).astype(np.float16)
        return np.ascontiguousarray(
            wt.reshape(NREL, 4, P, HID).transpose(2, 0, 1, 3)
            .reshape(P, NREL * 4 * HID))
    wlt = _pmaj_w(Wl5)
    wrt = _pmaj_w(Wr5)

    Wtab = np.concatenate([np.asarray(Wmu_l, f64), np.asarray(Wlv_l, f64)], 0)
    Wself = np.concatenate([np.asarray(Wmu_r, f64), np.asarray(Wlv_r, f64)], 0)
    Wall = np.concatenate([Wtab * s[None, :], Wself * s[None, :]], 0)  # [1024, 2560]
    # wallt[p, r*1024 + j] = Wall.T[r*128+p, j]
    wallt = np.ascontiguousarray(
        Wall.T.astype(np.float16).reshape(20, P, 1024).transpose(1, 0, 2)
        .reshape(P, 20 * 1024))

    tW = (Wtab @ t).astype(np.float32)                                  # [512]
    bself = (Wself @ t + np.concatenate(
        [np.asarray(bmu, f64), np.asarray(blv, f64)])).astype(np.float32)
    pbias = np.ascontiguousarray(
        np.tile(np.concatenate([tW, bself])[None, :], (P, 1)))  # [128, 1024]
    bsb = np.ascontiguousarray(bself.reshape(4, P).T)           # [128, 4]

    blb = np.ascontiguousarray(
        np.asarray(bl5, np.float32).reshape(NREL * 4, P).T)     # [128, 20]

    meta = (tuple(win1), tuple(win2))
    in_maps = []
    for c in range(NCORES):
        in_maps.append({
            "xtab": xtab, "xt": xt[c],
            "a1i": np.ascontiguousarray(a1i[c]),
            "a1v": np.ascontiguousarray(a1v[c]),
            "a2i": np.ascontiguousarray(a2i[c]),
            "a2v": np.ascontiguousarray(a2v[c]),
            "wlt": wlt, "wrt": wrt, "wallt": wallt,
            "blb": blb, "pbias": pbias, "bsb": bsb,
        })
    return meta, in_maps, pos


# ----------------------------------------------------------------------------
# Device kernel
# ----------------------------------------------------------------------------

def _build(meta):
    import concourse.bacc as bacc
    import concourse.bass as bass
    import concourse.tile as tile
    import concourse.mybir as mybir

    win1, win2 = meta
    nch1 = [len(w) for w in win1]          # 25 cells, (g, k) order
    base1 = np.concatenate([[0], np.cumsum(nch1)[:-1]])
    C1 = int(np.sum(nch1))
    nch2 = [len(w) for w in win2]          # 25 cells, (s, d) order
    base2 = np.concatenate([[0], np.cumsum(nch2)[:-1]])
    C2 = int(np.sum(nch2))

    f16, f32, i32 = mybir.dt.float16, mybir.dt.float32, mybir.dt.int32

    nc = bacc.Bacc("TRN2", target_bir_lowering=False, debug=False,
                   num_devices=NCORES)

    xtab_t = nc.dram_tensor("xtab", [N, IN], f16, kind="ExternalInput")
    xt_t = nc.dram_tensor("xt", [P, NG * 2048], f16, kind="ExternalInput")
    a1i_t = nc.dram_tensor("a1i", [P, C1], i32, kind="ExternalInput")
    a1v_t = nc.dram_tensor("a1v", [P, 25 * 512], f16, kind="ExternalInput")
    a2i_t = nc.dram_tensor("a2i", [P, C2], i32, kind="ExternalInput")
    a2v_t = nc.dram_tensor("a2v", [P, 25 * 512], f16, kind="ExternalInput")
    wlt_t = nc.dram_tensor("wlt", [P, NREL * 2048], f16, kind="ExternalInput")
    wrt_t = nc.dram_tensor("wrt", [P, NREL * 2048], f16, kind="ExternalInput")
    wallt_t = nc.dram_tensor("wallt", [P, 20 * 1024], f16, kind="ExternalInput")
    blb_t = nc.dram_tensor("blb", [P, NREL * 4], f32, kind="ExternalInput")
    pbias_t = nc.dram_tensor("pbias", [P, 1024], f32, kind="ExternalInput")
    bsb_t = nc.dram_tensor("bsb", [P, 4], f32, kind="ExternalInput")
    out_t = nc.dram_tensor("out", [P, NG * 2048], f16, kind="ExternalOutput")

    warm_t = nc.dram_tensor("warm", [P, 512], f16, kind="Internal")
    ag_in = nc.dram_tensor("ag_in", [NPAD, 512], f16, kind="Internal")
    ag_tabs = [nc.dram_tensor(f"ag_tab{s}", [NCORES * 512, 512], f16,
                              kind="Internal", addr_space="Shared")
               for s in range(NG)]

    with tile.TileContext(nc) as tc:
        with (
            tc.tile_pool(name="constp", bufs=1) as constp,
            tc.tile_pool(name="resp", bufs=1) as resp,
            tc.tile_pool(name="iop", bufs=3) as iop,
            tc.tile_pool(name="actp", bufs=2) as actp,
            tc.tile_pool(name="psum", bufs=4, space="PSUM") as pp,
        ):
            # ---- constants / index tiles ----
            blb_sb = constp.tile([P, NREL * 4], f32, name="blb_sb", tag="blb")
            nc.sync.dma_start(out=blb_sb[:], in_=blb_t.ap())
            pbias_sb = constp.tile([P, 1024], f32, name="pbias_sb", tag="pb")
            nc.sync.dma_start(out=pbias_sb[:], in_=pbias_t.ap())
            bsb_sb = constp.tile([P, 4], f32, name="bsb_sb", tag="bsb")
            nc.sync.dma_start(out=bsb_sb[:], in_=bsb_t.ap())

            idx1_sb = resp.tile([P, C1], i32, name="idx1_sb", tag="idx1")
            nc.gpsimd.dma_start(out=idx1_sb[:], in_=a1i_t.ap())
            idx2_sb = resp.tile([P, C2], i32, name="idx2_sb", tag="idx2")
            nc.gpsimd.dma_start(out=idx2_sb[:], in_=a2i_t.ap())

            # ---- resident tensors ----
            wl_sb = resp.tile([P, NREL * 2048], f16, name="wl_sb", tag="wl")
            wr_sb = resp.tile([P, NREL * 2048], f16, name="wr_sb", tag="wr")
            wall_sb = resp.tile([P, 20 * 1024], f16, name="wall_sb", tag="wall")
            sf_sb = resp.tile([P, 20 * 512], f16, name="sf_sb", tag="sf")
            acc_sb = resp.tile([P, NG * 2048], f16, name="acc_sb", tag="acc")

            def fetch_xt(g):
                xtg = iop.tile([P, 2048], f16, name=f"xt_{g}", tag="xt",
                               bufs=3)
                nc.scalar.dma_start(
                    out=xtg[:], in_=xt_t.ap()[:, g * 2048:(g + 1) * 2048])
                return xtg

            # spread the input streams: xt on scalar; wall halves are
            # emitted after the wl/wr streams below
            xtg0 = fetch_xt(0)

            # ---------------- helpers ----------------
            def fetch_av1(g, k):
                cell = g * NREL + k
                av = iop.tile([P, 512], f16, name=f"a1_{g}_{k}", tag="av1",
                              bufs=6)
                nc.sync.dma_start(
                    out=av[:], in_=a1v_t.ap()[:, cell * 512:(cell + 1) * 512])
                return av

            def fetch_av2(s, d):
                cell = s * NG + d
                av = iop.tile([P, 512], f16, name=f"a2_{s}_{d}", tag="av2",
                              bufs=6)
                nc.sync.dma_start(
                    out=av[:], in_=a2v_t.ap()[:, cell * 512:(cell + 1) * 512])
                return av

            # group-0 one-hot values first on sync, then the weights
            # (wl on sync, wr on scalar, so the streams race the dense MMs)
            avs1 = [fetch_av1(0, k) for k in range(NREL)]
            for k in range(NREL):
                nc.sync.dma_start(
                    out=wl_sb[:, k * 2048:(k + 1) * 2048],
                    in_=wlt_t.ap()[:, k * 2048:(k + 1) * 2048])
                nc.scalar.dma_start(
                    out=wr_sb[:, k * 2048:(k + 1) * 2048],
                    in_=wrt_t.ap()[:, k * 2048:(k + 1) * 2048])
            for r in range(20):
                nc.sync.dma_start(
                    out=wall_sb[:, r * 1024:r * 1024 + 512],
                    in_=wallt_t.ap()[:, r * 1024:r * 1024 + 512])
            for r in range(20):
                nc.sync.dma_start(
                    out=wall_sb[:, r * 1024 + 512:(r + 1) * 1024],
                    in_=wallt_t.ap()[:, r * 1024 + 512:(r + 1) * 1024])

            # ---- PE warm-up while the first gathers stream in ----
            wu = constp.tile([P, 512], f16, name="wu", tag="wu")
            nc.vector.memset(wu[:], 0.0)
            wu_ps = pp.tile([P, 512], f32, space="PSUM", name="wu_ps",
                            tag="sm", bufs=4)
            for i in range(32):
                nc.tensor.matmul(out=wu_ps[:], lhsT=wu[:, 0:P], rhs=wu[:],
                                 start=(i == 0), stop=(i == 31))
            nc.vector.tensor_copy(out=wu[:], in_=wu_ps[:])
            nc.scalar.dma_start(out=warm_t.ap(), in_=wu[:])
            def l1_gathers(g, k):
                cell = g * NREL + k
                cbase = int(base1[cell])
                gts = []
                for ci in range(nch1[cell]):
                    gth = iop.tile([P, 512], f16, name=f"g1_{g}_{k}_{ci}",
                                   tag="gth", bufs=24)
                    nc.gpsimd.indirect_dma_start(
                        out=gth[:], out_offset=None,
                        in_=xtab_t.ap(),
                        in_offset=bass.IndirectOffsetOnAxis(
                            ap=idx1_sb[:, cbase + ci:cbase + ci + 1], axis=0))
                    gts.append(gth)
                return gts

            def l2_gathers(s, d):
                cell = s * NG + d
                cbase = int(base2[cell])
                gts = []
                for ci in range(nch2[cell]):
                    gth = iop.tile([P, 512], f16, name=f"g2_{s}_{d}_{ci}",
                                   tag="gth", bufs=24)
                    nc.gpsimd.indirect_dma_start(
                        out=gth[:], out_offset=None,
                        in_=ag_tabs[s].ap(),
                        in_offset=bass.IndirectOffsetOnAxis(
                            ap=idx2_sb[:, cbase + ci:cbase + ci + 1], axis=0))
                    gts.append(gth)
                return gts

            def l1_agg(g, k, gts, av):
                """aggregation for cell (g, k) -> mean tiles (Act copies)"""
                cell = g * NREL + k
                aggs = [pp.tile([P, 1024], f32, space="PSUM",
                                name=f"agg_{g}_{k}_{h}", tag="big", bufs=2)
                        for h in range(2)]
                for cc in range(4):
                    tgt = aggs[cc // 2]
                    off = (cc % 2) * 512
                    for ci, (lo, hi) in enumerate(win1[cell]):
                        nc.tensor.matmul(
                            out=tgt[:, off + lo:off + hi],
                            lhsT=gts[ci][:, cc * P:(cc + 1) * P],
                            rhs=av[:, lo:hi], start=True, stop=True)
                means = []
                for kc in range(4):
                    m = actp.tile([P, 512], f16, name=f"mean_{g}_{k}_{kc}",
                                  tag=f"mean{kc}")
                    nc.scalar.copy(
                        out=m[:],
                        in_=aggs[kc // 2][:, (kc % 2) * 512:
                                          (kc % 2) * 512 + 512])
                    means.append(m)
                return means

            def l1_dense(g, k, means, xtg):
                """dense h = relu(Wl@mean + Wr@x + b) for cell (g, k)"""
                rt = actp.tile([P, 2048], f16, name=f"rt_{g}_{k}",
                               tag=f"rt{k}", bufs=1)
                for mc in range(4):
                    h_ps = pp.tile([P, 512], f32, space="PSUM",
                                   name=f"h_{g}_{k}_{mc}", tag="sm", bufs=4)
                    for kc in range(4):
                        nc.tensor.matmul(
                            out=h_ps[:],
                            lhsT=wl_sb[:, k * 2048 + kc * 512 + mc * P:
                                       k * 2048 + kc * 512 + (mc + 1) * P],
                            rhs=means[kc][:], start=(kc == 0), stop=False)
                    for kc in range(4):
                        nc.tensor.matmul(
                            out=h_ps[:],
                            lhsT=wr_sb[:, k * 2048 + kc * 512 + mc * P:
                                       k * 2048 + kc * 512 + (mc + 1) * P],
                            rhs=xtg[:, kc * 512:(kc + 1) * 512],
                            start=False, stop=(kc == 3))
                    nc.vector.tensor_scalar(
                        out=rt[:, mc * 512:(mc + 1) * 512], in0=h_ps[:],
                        scalar1=blb_sb[:, k * 4 + mc:k * 4 + mc + 1],
                        scalar2=0.0,
                        op0=mybir.AluOpType.add, op1=mybir.AluOpType.max)
                return rt

            def l2_cell(s, d, gts, av):
                """one (src-group, dst-group) partial-sum pass into acc.
                Feature-major: out[tab-chunk cc, node cols]."""
                cell = s * NG + d
                pss = [pp.tile([P, 1024], f32, space="PSUM",
                               name=f"m2_{s}_{d}_{h}", tag="big", bufs=2)
                       for h in range(2)]
                for cc in range(4):
                    tgt = pss[cc // 2]
                    off = (cc % 2) * 512
                    for ci, (lo, hi) in enumerate(win2[cell]):
                        nc.tensor.matmul(
                            out=tgt[:, off + lo:off + hi],
                            lhsT=gts[ci][:, cc * P:(cc + 1) * P],
                            rhs=av[:, lo:hi], start=True, stop=True)
                for h in range(2):
                    a = acc_sb[:, d * 2048 + h * 1024:d * 2048 + h * 1024 + 1024]
                    if s == 0:
                        nc.vector.tensor_copy(out=a, in_=pss[h][:])
                    else:
                        nc.vector.tensor_tensor(out=a, in0=pss[h][:], in1=a,
                                                op=mybir.AluOpType.add)

            def proj_tab(g, rts):
                """node-major tab projection for group g -> ag_in rows"""
                for ncx in range(4):
                    ps = pp.tile([P, 512], f32, space="PSUM",
                                 name=f"pt_{g}_{ncx}", tag="sm", bufs=4)
                    for r in range(20):
                        nc.tensor.matmul(
                            out=ps[:],
                            lhsT=rts[r // 4][:, (r % 4) * 512 + ncx * P:
                                             (r % 4) * 512 + (ncx + 1) * P],
                            rhs=wall_sb[:, r * 1024:r * 1024 + 512],
                            start=(r == 0), stop=(r == 19))
                    agin = actp.tile([P, 512], f16,
                                     name=f"agin_{g}_{ncx}", tag="agin", bufs=1)
                    nc.vector.tensor_tensor(
                        out=agin[:], in0=ps[:], in1=pbias_sb[:, 0:512],
                        op=mybir.AluOpType.add)
                    nc.sync.dma_start(
                        out=ag_in.ap()[g * 512 + ncx * P:
                                       g * 512 + (ncx + 1) * P, :],
                        in_=agin[:])

            def proj_self(g, rts):
                """feature-major self projection for group g -> sf slices"""
                for sc in range(4):
                    ps = pp.tile([P, 512], f32, space="PSUM",
                                 name=f"psf_{g}_{sc}", tag="sm", bufs=4)
                    for r in range(20):
                        nc.tensor.matmul(
                            out=ps[:],
                            lhsT=wall_sb[:, r * 1024 + 512 + sc * P:
                                         r * 1024 + 512 + (sc + 1) * P],
                            rhs=rts[r // 4][:, (r % 4) * 512:
                                            (r % 4 + 1) * 512],
                            start=(r == 0), stop=(r == 19))
                    nc.vector.tensor_scalar(
                        out=sf_sb[:, (g * 4 + sc) * 512:
                                  (g * 4 + sc + 1) * 512],
                        in0=ps[:], scalar1=bsb_sb[:, sc:sc + 1],
                        scalar2=None, op0=mybir.AluOpType.add)

            def emit_ag(g):
                nc.gpsimd.collective_compute(
                    "AllGather", mybir.AluOpType.bypass,
                    replica_groups=[list(range(NCORES))],
                    ins=[ag_in.ap()[g * 512:(g + 1) * 512, :]],
                    outs=[ag_tabs[g].ap()])

            def combine(d):
                for cc in range(4):
                    ob = actp.tile([P, 512], f16, name=f"ob_{d}_{cc}",
                                   tag="ob", bufs=4)
                    nc.vector.tensor_tensor(
                        out=ob[:],
                        in0=acc_sb[:, d * 2048 + cc * 512:
                                   d * 2048 + (cc + 1) * 512],
                        in1=sf_sb[:, (d * 4 + cc) * 512:
                                  (d * 4 + cc + 1) * 512],
                        op=mybir.AluOpType.add)
                    q = [nc.sync, nc.scalar, nc.gpsimd][(d * 4 + cc) % 3]
                    q.dma_start(
                        out=out_t.ap()[:, d * 2048 + cc * 512:
                                       d * 2048 + (cc + 1) * 512],
                        in_=ob[:])

            # ---------------- main schedule ----------------
            all_rts = {}
            xtg = xtg0
            for g in range(NG):
                if g > 0:
                    avs1 = [fetch_av1(g, k) for k in range(NREL)]
                    xtg = fetch_xt(g)
                avs2 = [fetch_av2(g - 2, d) for d in range(NG)] \
                    if g >= 2 else None
                rts = []
                prev = None
                for k in range(NREL):
                    # gathers first (gpsimd queue), then compute
                    gts1 = l1_gathers(g, k)
                    gts2 = l2_gathers(g - 2, k) if g >= 2 else None
                    # AllGather of the previous group, emitted mid-stream
                    # once its input is surely written
                    if k == 2 and g >= 1:
                        emit_ag(g - 1)
                    # software pipeline: agg(k) and the layer-2 pass run
                    # while cell k-1's mean copies finish on Act
                    means = l1_agg(g, k, gts1, avs1[k])
                    if gts2 is not None:
                        l2_cell(g - 2, k, gts2, avs2[k])
                    if prev is not None:
                        rts.append(l1_dense(g, prev[0], prev[1], xtg))
                    prev = (k, means)
                rts.append(l1_dense(g, prev[0], prev[1], xtg))
                # group 4's self projection is deferred into the AG_4 window
                proj_tab(g, rts)
                if g < NG - 1:
                    proj_self(g, rts)
                else:
                    all_rts[g] = rts
            # tail: AG_4 fires as soon as its agin lands; the deferred
            # self projection and s=3 pass hide its wire time
            emit_ag(NG - 1)
            avt2 = [fetch_av2(3, d) for d in range(NG)]
            tail2 = [l2_gathers(3, d) for d in range(NG)]
            avt3 = [fetch_av2(4, d) for d in range(NG)]
            tail3 = [l2_gathers(4, d) for d in range(NG)]
            proj_self(4, all_rts[4])
            for d in range(NG):
                l2_cell(3, d, tail2[d], avt2[d])
            for d in range(NG):
                l2_cell(4, d, tail3[d], avt3[d])
                combine(d)

    nc.compile()
    return nc


# ----------------------------------------------------------------------------
# Entry point
# ----------------------------------------------------------------------------

_CACHE = {}


def build_and_run(inputs, trace=False, trace_kwargs=None):
    from concourse import bass_utils

    meta, in_maps, pos = _preprocess(**inputs)
    if meta not in _CACHE:
        _CACHE[meta] = _build(meta)
    nc = _CACHE[meta]
    res = bass_utils.run_bass_kernel_spmd(
        nc, in_maps, core_ids=list(range(NCORES)),
        trace=trace, **(trace_kwargs or {}))

    mu = np.empty((N, OUT), np.float32)
    lv = np.empty((N, OUT), np.float32)
    for c in range(NCORES):
        raw = np.asarray(res.results[c]["out"], np.float32)
        blk = raw.reshape(P, NG, 4, 512).transpose(2, 0, 1, 3).reshape(
            512, NPAD)                          # [512 ch, 2560 positions]
        ids = np.arange(c * NLOC, (c + 1) * NLOC)
        mu[ids] = blk[0:OUT, pos[ids]].T
        lv[ids] = blk[OUT:2 * OUT, pos[ids]].T
    return (mu, lv), res


def kernel(**inputs):
    out, _ = build_and_run(inputs, trace=False)
    return out


# revision 54
# speedup vs baseline: 1.0307x; 1.0307x over previous
"""Trainium2 Bass kernel for a 2-layer relational GraphSAGE VGAE encoder.

Contract: kernel(**inputs) takes the FULL unsharded inputs (as produced by
setup_inputs()) and returns the full (mu, logvar) tuple.

Strategy (8 NeuronCores, SPMD single NEFF):
  - Nodes block-sharded: core c owns nodes [c*2500, (c+1)*2500), relabeled
    within the core by descending out-degree and padded to 2560 positions
    (5 groups of 512).
  - Segment-mean is a sequence of (gather 128 src rows) @ (one-hot 1/cnt)
    matmuls. Edges of a cell are bucketed into disjoint destination-column
    windows, each window small enough that every core has <=128 edges in it;
    one narrow matmul per (window, feature-chunk) writes its own psum slice
    with start=stop=True, so the whole cell costs only ~512 output columns
    per feature chunk instead of 512 per 128-edge chunk.
  - Layer-1 dense and the layer-2 projections are fp16 matmuls with fp32
    PSUM. BatchNorm (eval) is folded into the layer-2 weights on the host.
    The layer-2 projection is computed node-major (lhsT = h chunks, rhs =
    stacked [tab|self] weight block, 1024 wide), which both halves the
    LdWeights count and directly emits rows for the AllGather - no PE
    transposes anywhere.
  - The projected table rows are AllGather'd per node group (fp16) into 5
    shared tables (one per source group). Layer-2 aggregation is split by
    source group; the pass for source group s is interleaved into the
    layer-1 work of group s+2, so its gathers overlap compute and partial
    sums accumulate in an SBUF fp16 accumulator. Group 4 holds the
    lowest-out-degree nodes, keeping the post-AllGather tail short.
"""
import sys

sys.path.insert(0, "/opt/trn_rl_repo")

import numpy as np

NCORES = 8
N = 20000
E = 100000
IN = 512
HID = 512
CAT = 2560
OUT = 256
BN_EPS = 1e-5

NLOC = N // NCORES          # 2500
NPAD = 2560                 # 5 * 512
NG = NPAD // 512            # 5 node groups of 512 per core
NREL = 5
P = 128


# ----------------------------------------------------------------------------
# Host-side preprocessing: relabeling, window chunking, weight folding
# ----------------------------------------------------------------------------

def _windows(counts, force):
    """Split [0,512) into consecutive windows such that every core has
    <=128 edges per window.  counts: [NCORES, 512] per-core per-col edge
    counts.  force: cols where a boundary is mandatory."""
    wins = []
    lo = 0
    run = np.zeros(NCORES, np.int64)
    for col in range(512):
        c = counts[:, col]
        assert (c <= P).all(), "single column exceeds 128 edges on a core"
        if col > lo and ((run + c > P).any() or col in force):
            wins.append((lo, col))
            lo = col
            run = np.zeros(NCORES, np.int64)
        run += c
    wins.append((lo, 512))
    return wins


def _pack_cell(e, dcore, dcol, idxval, aval, wins):
    """Pack a cell's edges into per-window chunks.

    Returns idxs [nwin, NCORES, P] int32 and av [NCORES, P, 512] f16."""
    nwin = len(wins)
    los = np.array([w[0] for w in wins])
    av = np.zeros((NCORES, P, 512), np.float16)
    idxs = np.zeros((nwin, NCORES, P), np.int32)
    if len(e) == 0:
        return idxs, av
    wi = np.searchsorted(los, dcol[e], side="right") - 1
    key = dcore[e] * nwin + wi
    order = np.argsort(key, kind="stable")
    ke = key[order]
    first = np.r_[True, ke[1:] != ke[:-1]]
    runstart = np.flatnonzero(first)
    rid = np.cumsum(first) - 1
    rowp = np.arange(len(ke)) - runstart[rid]
    assert (rowp < P).all()
    eo = e[order]
    cc = ke // nwin
    ww = ke % nwin
    idxs[ww, cc, rowp] = idxval[eo]
    av[cc, rowp, dcol[eo]] = aval[eo]
    return idxs, av


def _preprocess(x, edge_index, edge_attr, Wl5, Wr5, bl5,
                Wmu_l, Wmu_r, bmu, Wlv_l, Wlv_r, blv,
                gamma, beta, run_mean, run_var):
    x = np.asarray(x, np.float32)
    src = np.asarray(edge_index[0], np.int64)
    dst = np.asarray(edge_index[1], np.int64)
    rel = np.asarray(edge_attr, np.int64)

    # --- relabel nodes within each core by descending out-degree ---
    outdeg = np.bincount(src, minlength=N)
    pos = np.empty(N, np.int64)
    for c in range(NCORES):
        ids = np.arange(c * NLOC, (c + 1) * NLOC)
        order = ids[np.argsort(-outdeg[ids], kind="stable")]
        pos[order] = np.arange(NLOC)

    cnt1 = np.bincount(rel * N + dst, minlength=NREL * N).reshape(NREL, N)
    inv1 = (1.0 / np.maximum(cnt1, 1.0)).astype(np.float32)
    cnt2 = np.bincount(dst, minlength=N)
    inv2 = (1.0 / np.maximum(cnt2, 1.0)).astype(np.float32)

    dcore = dst // NLOC
    dpos = pos[dst]
    dgrp = dpos // 512
    dcol = dpos % 512
    spos = pos[src]
    sgrp = spos // 512
    srow = ((src // NLOC) * 512 + spos % 512).astype(np.int32)

    # --- L1 cells: (group, rel) in program order ---
    win1 = []
    i1 = []
    v1 = []
    v1e = inv1[rel, dst].astype(np.float32)
    for g in range(NG):
        for k in range(NREL):
            e = np.flatnonzero((dgrp == g) & (rel == k))
            cnt = np.zeros((NCORES, 512), np.int64)
            np.add.at(cnt, (dcore[e], dcol[e]), 1)
            wins = _windows(cnt, ())
            win1.append(tuple(wins))
            idxs, av = _pack_cell(e, dcore, dcol, src.astype(np.int32),
                                  v1e, wins)
            i1.append(idxs)
            v1.append(av)
    C1 = sum(len(w) for w in win1)
    a1i = np.concatenate(i1, axis=0).transpose(1, 2, 0)          # [NC, P, C1]
    a1v = np.stack(v1, axis=0).transpose(1, 2, 0, 3).reshape(
        NCORES, P, len(v1) * 512)                                # [NC, P, 25*512]

    # --- L2 cells: (src-group, dst-group) in program order ---
    win2 = []
    i2 = []
    v2 = []
    v2e = inv2[dst].astype(np.float32)
    for s in range(NG):
        for d in range(NG):
            e = np.flatnonzero((dgrp == d) & (sgrp == s))
            cnt = np.zeros((NCORES, 512), np.int64)
            np.add.at(cnt, (dcore[e], dcol[e]), 1)
            wins = _windows(cnt, ())
            win2.append(tuple(wins))
            idxs, av = _pack_cell(e, dcore, dcol, srow, v2e, wins)
            i2.append(idxs)
            v2.append(av)
    C2 = sum(len(w) for w in win2)
    a2i = np.concatenate(i2, axis=0).transpose(1, 2, 0)          # [NC, P, C2]
    a2v = np.stack(v2, axis=0).transpose(1, 2, 0, 3).reshape(
        NCORES, P, len(v2) * 512)

    # --- node features ---
    xtab = x.astype(np.float16)                                  # [N, 512]
    xt = np.zeros((NCORES, P, NG * 2048), np.float16)
    for c in range(NCORES):
        ids = np.arange(c * NLOC, (c + 1) * NLOC)
        arr = np.zeros((NPAD, IN), np.float32)
        arr[pos[ids]] = x[ids]
        # xt[c][p, g*2048 + kc*512 + col] = arr[g*512 + col, kc*128 + p]
        xt[c] = np.ascontiguousarray(
            arr.T.reshape(4, P, NG, 512).transpose(1, 2, 0, 3)
            .reshape(P, NG * 2048)).astype(np.float16)

    # --- weight folding (BN eval folded into layer-2 weights) ---
    f64 = np.float64
    s = np.asarray(gamma, f64) / np.sqrt(np.asarray(run_var, f64) + BN_EPS)
    t = np.asarray(beta, f64) - np.asarray(run_mean, f64) * s

    # wl/wr: [P, k*2048 + kc*512 + j] = Wx[k].T[kc*128+p, j]
    def _pmaj_w(W5):
        wt = np.asarray(W5, np.float32).transpose(0, 2, 1).astype(np.float16)
        return np.ascontiguousarray(
            wt.reshape(NREL, 4, P, HID).transpose(2, 0, 1, 3)
            .reshape(P, NREL * 4 * HID))
    wlt = _pmaj_w(Wl5)
    wrt = _pmaj_w(Wr5)

    Wtab = np.concatenate([np.asarray(Wmu_l, f64), np.asarray(Wlv_l, f64)], 0)
    Wself = np.concatenate([np.asarray(Wmu_r, f64), np.asarray(Wlv_r, f64)], 0)
    Wall = np.concatenate([Wtab * s[None, :], Wself * s[None, :]], 0)  # [1024, 2560]
    # wallt[p, r*1024 + j] = Wall.T[r*128+p, j]
    wallt = np.ascontiguousarray(
        Wall.T.astype(np.float16).reshape(20, P, 1024).transpose(1, 0, 2)
        .reshape(P, 20 * 1024))

    tW = (Wtab @ t).astype(np.float32)                                  # [512]
    bself = (Wself @ t + np.concatenate(
        [np.asarray(bmu, f64), np.asarray(blv, f64)])).astype(np.float32)
    pbias = np.ascontiguousarray(
        np.tile(np.concatenate([tW, bself])[None, :], (P, 1)))  # [128, 1024]
    bsb = np.ascontiguousarray(bself.reshape(4, P).T)           # [128, 4]

    blb = np.ascontiguousarray(
        np.asarray(bl5, np.float32).reshape(NREL * 4, P).T)     # [128, 20]

    meta = (tuple(win1), tuple(win2))
    in_maps = []
    for c in range(NCORES):
        in_maps.append({
            "xtab": xtab, "xt": xt[c],
            "a1i": np.ascontiguousarray(a1i[c]),
            "a1v": np.ascontiguousarray(a1v[c]),
            "a2i": np.ascontiguousarray(a2i[c]),
            "a2v": np.ascontiguousarray(a2v[c]),
            "wlt": wlt, "wrt": wrt, "wallt": wallt,
            "blb": blb, "pbias": pbias, "bsb": bsb,
        })
    return meta, in_maps, pos


# ----------------------------------------------------------------------------
# Device kernel
# ----------------------------------------------------------------------------

def _build(meta):
    import concourse.bacc as bacc
    import concourse.bass as bass
    import concourse.tile as tile
    import concourse.mybir as mybir

    win1, win2 = meta
    nch1 = [len(w) for w in win1]          # 25 cells, (g, k) order
    base1 = np.concatenate([[0], np.cumsum(nch1)[:-1]])
    C1 = int(np.sum(nch1))
    nch2 = [len(w) for w in win2]          # 25 cells, (s, d) order
    base2 = np.concatenate([[0], np.cumsum(nch2)[:-1]])
    C2 = int(np.sum(nch2))

    f16, f32, i32 = mybir.dt.float16, mybir.dt.float32, mybir.dt.int32

    nc = bacc.Bacc("TRN2", target_bir_lowering=False, debug=False,
                   num_devices=NCORES)

    xtab_t = nc.dram_tensor("xtab", [N, IN], f16, kind="ExternalInput")
    xt_t = nc.dram_tensor("xt", [P, NG * 2048], f16, kind="ExternalInput")
    a1i_t = nc.dram_tensor("a1i", [P, C1], i32, kind="ExternalInput")
    a1v_t = nc.dram_tensor("a1v", [P, 25 * 512], f16, kind="ExternalInput")
    a2i_t = nc.dram_tensor("a2i", [P, C2], i32, kind="ExternalInput")
    a2v_t = nc.dram_tensor("a2v", [P, 25 * 512], f16, kind="ExternalInput")
    wlt_t = nc.dram_tensor("wlt", [P, NREL * 2048], f16, kind="ExternalInput")
    wrt_t = nc.dram_tensor("wrt", [P, NREL * 2048], f16, kind="ExternalInput")
    wallt_t = nc.dram_tensor("wallt", [P, 20 * 1024], f16, kind="ExternalInput")
    blb_t = nc.dram_tensor("blb", [P, NREL * 4], f32, kind="ExternalInput")
    pbias_t = nc.dram_tensor("pbias", [P, 1024], f32, kind="ExternalInput")
    bsb_t = nc.dram_tensor("bsb", [P, 4], f32, kind="ExternalInput")
    out_t = nc.dram_tensor("out", [P, NG * 2048], f16, kind="ExternalOutput")

    warm_t = nc.dram_tensor("warm", [P, 512], f16, kind="Internal")
    ag_in = nc.dram_tensor("ag_in", [NPAD, 512], f16, kind="Internal")
    ag_tabs = [nc.dram_tensor(f"ag_tab{s}", [NCORES * 512, 512], f16,
                              kind="Internal", addr_space="Shared")
               for s in range(NG)]

    with tile.TileContext(nc) as tc:
        with (
            tc.tile_pool(name="constp", bufs=1) as constp,
            tc.tile_pool(name="resp", bufs=1) as resp,
            tc.tile_pool(name="iop", bufs=3) as iop,
            tc.tile_pool(name="actp", bufs=2) as actp,
            tc.tile_pool(name="psum", bufs=4, space="PSUM") as pp,
        ):
            # ---- constants / index tiles ----
            blb_sb = constp.tile([P, NREL * 4], f32, name="blb_sb", tag="blb")
            nc.sync.dma_start(out=blb_sb[:], in_=blb_t.ap())
            pbias_sb = constp.tile([P, 1024], f32, name="pbias_sb", tag="pb")
            nc.sync.dma_start(out=pbias_sb[:], in_=pbias_t.ap())
            bsb_sb = constp.tile([P, 4], f32, name="bsb_sb", tag="bsb")
            nc.sync.dma_start(out=bsb_sb[:], in_=bsb_t.ap())

            idx1_sb = resp.tile([P, C1], i32, name="idx1_sb", tag="idx1")
            nc.gpsimd.dma_start(out=idx1_sb[:], in_=a1i_t.ap())
            idx2_sb = resp.tile([P, C2], i32, name="idx2_sb", tag="idx2")
            nc.gpsimd.dma_start(out=idx2_sb[:], in_=a2i_t.ap())

            # ---- resident tensors ----
            wl_sb = resp.tile([P, NREL * 2048], f16, name="wl_sb", tag="wl")
            wr_sb = resp.tile([P, NREL * 2048], f16, name="wr_sb", tag="wr")
            wall_sb = resp.tile([P, 20 * 1024], f16, name="wall_sb", tag="wall")
            sf_sb = resp.tile([P, 20 * 512], f16, name="sf_sb", tag="sf")
            acc_sb = resp.tile([P, NG * 2048], f16, name="acc_sb", tag="acc")

            def fetch_xt(g):
                xtg = iop.tile([P, 2048], f16, name=f"xt_{g}", tag="xt",
                               bufs=3)
                nc.scalar.dma_start(
                    out=xtg[:], in_=xt_t.ap()[:, g * 2048:(g + 1) * 2048])
                return xtg

            # spread the input streams: xt on scalar; wall halves are
            # emitted after the wl/wr streams below
            xtg0 = fetch_xt(0)

            # ---------------- helpers ----------------
            def fetch_av1(g, k):
                cell = g * NREL + k
                av = iop.tile([P, 512], f16, name=f"a1_{g}_{k}", tag="av1",
                              bufs=6)
                nc.sync.dma_start(
                    out=av[:], in_=a1v_t.ap()[:, cell * 512:(cell + 1) * 512])
                return av

            def fetch_av2(s, d):
                cell = s * NG + d
                av = iop.tile([P, 512], f16, name=f"a2_{s}_{d}", tag="av2",
                              bufs=6)
                nc.sync.dma_start(
                    out=av[:], in_=a2v_t.ap()[:, cell * 512:(cell + 1) * 512])
                return av

            # group-0 one-hot values first on sync, then the weights
            # (wl on sync, wr on scalar, so the streams race the dense MMs)
            avs1 = [fetch_av1(0, k) for k in range(NREL)]
            for k in range(NREL):
                nc.sync.dma_start(
                    out=wl_sb[:, k * 2048:(k + 1) * 2048],
                    in_=wlt_t.ap()[:, k * 2048:(k + 1) * 2048])
                nc.scalar.dma_start(
                    out=wr_sb[:, k * 2048:(k + 1) * 2048],
                    in_=wrt_t.ap()[:, k * 2048:(k + 1) * 2048])
            for r in range(20):
                nc.sync.dma_start(
                    out=wall_sb[:, r * 1024:r * 1024 + 512],
                    in_=wallt_t.ap()[:, r * 1024:r * 1024 + 512])
            for r in range(20):
                nc.sync.dma_start(
                    out=wall_sb[:, r * 1024 + 512:(r + 1) * 1024],
                    in_=wallt_t.ap()[:, r * 1024 + 512:(r + 1) * 1024])

            # ---- PE warm-up while the first gathers stream in ----
            wu = constp.tile([P, 512], f16, name="wu", tag="wu")
            nc.vector.memset(wu[:], 0.0)
            wu_ps = pp.tile([P, 512], f32, space="PSUM", name="wu_ps",
                            tag="sm", bufs=4)
            for i in range(32):
                nc.tensor.matmul(out=wu_ps[:], lhsT=wu[:, 0:P], rhs=wu[:],
                                 start=(i == 0), stop=(i == 31))
            nc.vector.tensor_copy(out=wu[:], in_=wu_ps[:])
            nc.scalar.dma_start(out=warm_t.ap(), in_=wu[:])
            def l1_gathers(g, k):
                cell = g * NREL + k
                cbase = int(base1[cell])
                gts = []
                for ci in range(nch1[cell]):
                    gth = iop.tile([P, 512], f16, name=f"g1_{g}_{k}_{ci}",
                                   tag="gth", bufs=24)
                    nc.gpsimd.indirect_dma_start(
                        out=gth[:], out_offset=None,
                        in_=xtab_t.ap(),
                        in_offset=bass.IndirectOffsetOnAxis(
                            ap=idx1_sb[:, cbase + ci:cbase + ci + 1], axis=0))
                    gts.append(gth)
                return gts

            def l2_gathers(s, d):
                cell = s * NG + d
                cbase = int(base2[cell])
                gts = []
                for ci in range(nch2[cell]):
                    gth = iop.tile([P, 512], f16, name=f"g2_{s}_{d}_{ci}",
                                   tag="gth", bufs=24)
                    nc.gpsimd.indirect_dma_start(
                        out=gth[:], out_offset=None,
                        in_=ag_tabs[s].ap(),
                        in_offset=bass.IndirectOffsetOnAxis(
                            ap=idx2_sb[:, cbase + ci:cbase + ci + 1], axis=0))
                    gts.append(gth)
                return gts

            def l1_agg(g, k, gts, av):
                """aggregation for cell (g, k) -> mean tiles (Act copies)"""
                cell = g * NREL + k
                aggs = [pp.tile([P, 1024], f32, space="PSUM",
                                name=f"agg_{g}_{k}_{h}", tag="big", bufs=2)
                        for h in range(2)]
                for cc in range(4):
                    tgt = aggs[cc // 2]
                    off = (cc % 2) * 512
                    for ci, (lo, hi) in enumerate(win1[cell]):
                        nc.tensor.matmul(
                            out=tgt[:, off + lo:off + hi],
                            lhsT=gts[ci][:, cc * P:(cc + 1) * P],
                            rhs=av[:, lo:hi], start=True, stop=True)
                means = []
                for kc in range(4):
                    m = actp.tile([P, 512], f16, name=f"mean_{g}_{k}_{kc}",
                                  tag=f"mean{kc}")
                    nc.scalar.copy(
                        out=m[:],
                        in_=aggs[kc // 2][:, (kc % 2) * 512:
                                          (kc % 2) * 512 + 512])
                    means.append(m)
                return means

            def l1_dense(g, k, means, xtg):
                """dense h = relu(Wl@mean + Wr@x + b) for cell (g, k)"""
                rt = actp.tile([P, 2048], f16, name=f"rt_{g}_{k}",
                               tag=f"rt{k}", bufs=1)
                for mc in range(4):
                    h_ps = pp.tile([P, 512], f32, space="PSUM",
                                   name=f"h_{g}_{k}_{mc}", tag="sm", bufs=4)
                    # self side first: it does not depend on the mean
                    # copies, which land on Act while these MMs run
                    for kc in range(4):
                        nc.tensor.matmul(
                            out=h_ps[:],
                            lhsT=wr_sb[:, k * 2048 + kc * 512 + mc * P:
                                       k * 2048 + kc * 512 + (mc + 1) * P],
                            rhs=xtg[:, kc * 512:(kc + 1) * 512],
                            start=(kc == 0), stop=False)
                    for kc in range(4):
                        nc.tensor.matmul(
                            out=h_ps[:],
                            lhsT=wl_sb[:, k * 2048 + kc * 512 + mc * P:
                                       k * 2048 + kc * 512 + (mc + 1) * P],
                            rhs=means[kc][:], start=False, stop=(kc == 3))
                    nc.vector.tensor_scalar(
                        out=rt[:, mc * 512:(mc + 1) * 512], in0=h_ps[:],
                        scalar1=blb_sb[:, k * 4 + mc:k * 4 + mc + 1],
                        scalar2=0.0,
                        op0=mybir.AluOpType.add, op1=mybir.AluOpType.max)
                return rt

            def l2_cell(s, d, gts, av):
                """one (src-group, dst-group) partial-sum pass into acc.
                Feature-major: out[tab-chunk cc, node cols]."""
                cell = s * NG + d
                pss = [pp.tile([P, 1024], f32, space="PSUM",
                               name=f"m2_{s}_{d}_{h}", tag="big", bufs=2)
                       for h in range(2)]
                for cc in range(4):
                    tgt = pss[cc // 2]
                    off = (cc % 2) * 512
                    for ci, (lo, hi) in enumerate(win2[cell]):
                        nc.tensor.matmul(
                            out=tgt[:, off + lo:off + hi],
                            lhsT=gts[ci][:, cc * P:(cc + 1) * P],
                            rhs=av[:, lo:hi], start=True, stop=True)
                for h in range(2):
                    a = acc_sb[:, d * 2048 + h * 1024:d * 2048 + h * 1024 + 1024]
                    if s == 0:
                        nc.vector.tensor_copy(out=a, in_=pss[h][:])
                    else:
                        nc.vector.tensor_tensor(out=a, in0=pss[h][:], in1=a,
                                                op=mybir.AluOpType.add)

            def proj_tab(g, rts):
                """node-major tab projection for group g -> ag_in rows"""
                for ncx in range(4):
                    ps = pp.tile([P, 512], f32, space="PSUM",
                                 name=f"pt_{g}_{ncx}", tag="sm", bufs=4)
                    for r in range(20):
                        nc.tensor.matmul(
                            out=ps[:],
                            lhsT=rts[r // 4][:, (r % 4) * 512 + ncx * P:
                                             (r % 4) * 512 + (ncx + 1) * P],
                            rhs=wall_sb[:, r * 1024:r * 1024 + 512],
                            start=(r == 0), stop=(r == 19))
                    agin = actp.tile([P, 512], f16,
                                     name=f"agin_{g}_{ncx}", tag="agin", bufs=1)
                    nc.vector.tensor_tensor(
                        out=agin[:], in0=ps[:], in1=pbias_sb[:, 0:512],
                        op=mybir.AluOpType.add)
                    nc.sync.dma_start(
                        out=ag_in.ap()[g * 512 + ncx * P:
                                       g * 512 + (ncx + 1) * P, :],
                        in_=agin[:])

            def proj_self(g, rts):
                """feature-major self projection for group g -> sf slices"""
                for sc in range(4):
                    ps = pp.tile([P, 512], f32, space="PSUM",
                                 name=f"psf_{g}_{sc}", tag="sm", bufs=4)
                    for r in range(20):
                        nc.tensor.matmul(
                            out=ps[:],
                            lhsT=wall_sb[:, r * 1024 + 512 + sc * P:
                                         r * 1024 + 512 + (sc + 1) * P],
                            rhs=rts[r // 4][:, (r % 4) * 512:
                                            (r % 4 + 1) * 512],
                            start=(r == 0), stop=(r == 19))
                    nc.vector.tensor_scalar(
                        out=sf_sb[:, (g * 4 + sc) * 512:
                                  (g * 4 + sc + 1) * 512],
                        in0=ps[:], scalar1=bsb_sb[:, sc:sc + 1],
                        scalar2=None, op0=mybir.AluOpType.add)

            def emit_ag(g):
                nc.gpsimd.collective_compute(
                    "AllGather", mybir.AluOpType.bypass,
                    replica_groups=[list(range(NCORES))],
                    ins=[ag_in.ap()[g * 512:(g + 1) * 512, :]],
                    outs=[ag_tabs[g].ap()])

            def combine(d):
                for cc in range(4):
                    ob = actp.tile([P, 512], f16, name=f"ob_{d}_{cc}",
                                   tag="ob", bufs=4)
                    nc.vector.tensor_tensor(
                        out=ob[:],
                        in0=acc_sb[:, d * 2048 + cc * 512:
                                   d * 2048 + (cc + 1) * 512],
                        in1=sf_sb[:, (d * 4 + cc) * 512:
                                  (d * 4 + cc + 1) * 512],
                        op=mybir.AluOpType.add)
                    q = [nc.sync, nc.scalar, nc.gpsimd][(d * 4 + cc) % 3]
                    q.dma_start(
                        out=out_t.ap()[:, d * 2048 + cc * 512:
                                       d * 2048 + (cc + 1) * 512],
                        in_=ob[:])

            # ---------------- main schedule ----------------
            all_rts = {}
            xtg = xtg0
            for g in range(NG):
                if g > 0:
                    avs1 = [fetch_av1(g, k) for k in range(NREL)]
                    xtg = fetch_xt(g)
                avs2 = [fetch_av2(g - 2, d) for d in range(NG)] \
                    if g >= 2 else None
                rts = []
                for k in range(NREL):
                    # gathers first (gpsimd queue), then compute
                    gts1 = l1_gathers(g, k)
                    gts2 = l2_gathers(g - 2, k) if g >= 2 else None
                    # AllGather of the previous group, emitted mid-stream
                    # once its input is surely written
                    if k == 2 and g >= 1:
                        emit_ag(g - 1)
                    means = l1_agg(g, k, gts1, avs1[k])
                    rts.append(l1_dense(g, k, means, xtg))
                    if gts2 is not None:
                        l2_cell(g - 2, k, gts2, avs2[k])
                # group 4's self projection is deferred into the AG_4 window
                proj_tab(g, rts)
                if g < NG - 1:
                    proj_self(g, rts)
                else:
                    all_rts[g] = rts
            # tail: AG_4 fires as soon as its agin lands; the deferred
            # self projection and s=3 pass hide its wire time
            emit_ag(NG - 1)
            avt2 = [fetch_av2(3, d) for d in range(NG)]
            tail2 = [l2_gathers(3, d) for d in range(NG)]
            avt3 = [fetch_av2(4, d) for d in range(NG)]
            tail3 = [l2_gathers(4, d) for d in range(NG)]
            proj_self(4, all_rts[4])
            for d in range(NG):
                l2_cell(3, d, tail2[d], avt2[d])
            for d in range(NG):
                l2_cell(4, d, tail3[d], avt3[d])
                combine(d)

    nc.compile()
    return nc


# ----------------------------------------------------------------------------
# Entry point
# ----------------------------------------------------------------------------

_CACHE = {}


def build_and_run(inputs, trace=False, trace_kwargs=None):
    from concourse import bass_utils

    meta, in_maps, pos = _preprocess(**inputs)
    if meta not in _CACHE:
        _CACHE[meta] = _build(meta)
    nc = _CACHE[meta]
    res = bass_utils.run_bass_kernel_spmd(
        nc, in_maps, core_ids=list(range(NCORES)),
        trace=trace, **(trace_kwargs or {}))

    mu = np.empty((N, OUT), np.float32)
    lv = np.empty((N, OUT), np.float32)
    for c in range(NCORES):
        raw = np.asarray(res.results[c]["out"], np.float32)
        blk = raw.reshape(P, NG, 4, 512).transpose(2, 0, 1, 3).reshape(
            512, NPAD)                          # [512 ch, 2560 positions]
        ids = np.arange(c * NLOC, (c + 1) * NLOC)
        mu[ids] = blk[0:OUT, pos[ids]].T
        lv[ids] = blk[OUT:2 * OUT, pos[ids]].T
    return (mu, lv), res


def kernel(**inputs):
    out, _ = build_and_run(inputs, trace=False)
    return out


# revision 55
# speedup vs baseline: 1.0606x; 1.0291x over previous
"""Trainium2 Bass kernel for a 2-layer relational GraphSAGE VGAE encoder.

Contract: kernel(**inputs) takes the FULL unsharded inputs (as produced by
setup_inputs()) and returns the full (mu, logvar) tuple.

Strategy (8 NeuronCores, SPMD single NEFF):
  - Nodes block-sharded: core c owns nodes [c*2500, (c+1)*2500), relabeled
    within the core by descending out-degree and padded to 2560 positions
    (5 groups of 512).
  - Segment-mean is a sequence of (gather 128 src rows) @ (one-hot 1/cnt)
    matmuls. Edges of a cell are bucketed into disjoint destination-column
    windows, each window small enough that every core has <=128 edges in it;
    one narrow matmul per (window, feature-chunk) writes its own psum slice
    with start=stop=True, so the whole cell costs only ~512 output columns
    per feature chunk instead of 512 per 128-edge chunk.
  - Layer-1 dense and the layer-2 projections are fp16 matmuls with fp32
    PSUM. BatchNorm (eval) is folded into the layer-2 weights on the host.
    The layer-2 projection is computed node-major (lhsT = h chunks, rhs =
    stacked [tab|self] weight block, 1024 wide), which both halves the
    LdWeights count and directly emits rows for the AllGather - no PE
    transposes anywhere.
  - The projected table rows are AllGather'd per node group (fp16) into 5
    shared tables (one per source group). Layer-2 aggregation is split by
    source group; the pass for source group s is interleaved into the
    layer-1 work of group s+2, so its gathers overlap compute and partial
    sums accumulate in an SBUF fp16 accumulator. Group 4 holds the
    lowest-out-degree nodes, keeping the post-AllGather tail short.
"""
import sys

sys.path.insert(0, "/opt/trn_rl_repo")

import numpy as np

NCORES = 8
N = 20000
E = 100000
IN = 512
HID = 512
CAT = 2560
OUT = 256
BN_EPS = 1e-5

NLOC = N // NCORES          # 2500
NPAD = 2560                 # 5 * 512
NG = NPAD // 512            # 5 node groups of 512 per core
NREL = 5
P = 128


# ----------------------------------------------------------------------------
# Host-side preprocessing: relabeling, window chunking, weight folding
# ----------------------------------------------------------------------------

def _windows(counts, force):
    """Split [0,512) into consecutive windows such that every core has
    <=128 edges per window.  counts: [NCORES, 512] per-core per-col edge
    counts.  force: cols where a boundary is mandatory."""
    wins = []
    lo = 0
    run = np.zeros(NCORES, np.int64)
    for col in range(512):
        c = counts[:, col]
        assert (c <= P).all(), "single column exceeds 128 edges on a core"
        if col > lo and ((run + c > P).any() or col in force):
            wins.append((lo, col))
            lo = col
            run = np.zeros(NCORES, np.int64)
        run += c
    wins.append((lo, 512))
    return wins


def _pack_cell(e, dcore, dcol, idxval, aval, wins):
    """Pack a cell's edges into per-window chunks.

    Returns idxs [nwin, NCORES, P] int32 and av [NCORES, P, 512] f16."""
    nwin = len(wins)
    los = np.array([w[0] for w in wins])
    av = np.zeros((NCORES, P, 512), np.float16)
    idxs = np.zeros((nwin, NCORES, P), np.int32)
    if len(e) == 0:
        return idxs, av
    wi = np.searchsorted(los, dcol[e], side="right") - 1
    key = dcore[e] * nwin + wi
    order = np.argsort(key, kind="stable")
    ke = key[order]
    first = np.r_[True, ke[1:] != ke[:-1]]
    runstart = np.flatnonzero(first)
    rid = np.cumsum(first) - 1
    rowp = np.arange(len(ke)) - runstart[rid]
    assert (rowp < P).all()
    eo = e[order]
    cc = ke // nwin
    ww = ke % nwin
    idxs[ww, cc, rowp] = idxval[eo]
    av[cc, rowp, dcol[eo]] = aval[eo]
    return idxs, av


def _preprocess(x, edge_index, edge_attr, Wl5, Wr5, bl5,
                Wmu_l, Wmu_r, bmu, Wlv_l, Wlv_r, blv,
                gamma, beta, run_mean, run_var):
    x = np.asarray(x, np.float32)
    src = np.asarray(edge_index[0], np.int64)
    dst = np.asarray(edge_index[1], np.int64)
    rel = np.asarray(edge_attr, np.int64)

    # --- relabel nodes within each core by descending out-degree ---
    outdeg = np.bincount(src, minlength=N)
    pos = np.empty(N, np.int64)
    for c in range(NCORES):
        ids = np.arange(c * NLOC, (c + 1) * NLOC)
        order = ids[np.argsort(-outdeg[ids], kind="stable")]
        pos[order] = np.arange(NLOC)

    cnt1 = np.bincount(rel * N + dst, minlength=NREL * N).reshape(NREL, N)
    inv1 = (1.0 / np.maximum(cnt1, 1.0)).astype(np.float32)
    cnt2 = np.bincount(dst, minlength=N)
    inv2 = (1.0 / np.maximum(cnt2, 1.0)).astype(np.float32)

    dcore = dst // NLOC
    dpos = pos[dst]
    dgrp = dpos // 512
    dcol = dpos % 512
    spos = pos[src]
    sgrp = spos // 512
    srow = ((src // NLOC) * 512 + spos % 512).astype(np.int32)

    # --- L1 cells: (group, rel) in program order ---
    win1 = []
    i1 = []
    v1 = []
    v1e = inv1[rel, dst].astype(np.float32)
    for g in range(NG):
        for k in range(NREL):
            e = np.flatnonzero((dgrp == g) & (rel == k))
            cnt = np.zeros((NCORES, 512), np.int64)
            np.add.at(cnt, (dcore[e], dcol[e]), 1)
            wins = _windows(cnt, ())
            win1.append(tuple(wins))
            idxs, av = _pack_cell(e, dcore, dcol, src.astype(np.int32),
                                  v1e, wins)
            i1.append(idxs)
            v1.append(av)
    C1 = sum(len(w) for w in win1)
    a1i = np.concatenate(i1, axis=0).transpose(1, 2, 0)          # [NC, P, C1]
    a1v = np.stack(v1, axis=0).transpose(1, 2, 0, 3).reshape(
        NCORES, P, len(v1) * 512)                                # [NC, P, 25*512]

    # --- L2 cells: (src-group, dst-group) in program order ---
    win2 = []
    i2 = []
    v2 = []
    v2e = inv2[dst].astype(np.float32)
    for s in range(NG):
        for d in range(NG):
            e = np.flatnonzero((dgrp == d) & (sgrp == s))
            cnt = np.zeros((NCORES, 512), np.int64)
            np.add.at(cnt, (dcore[e], dcol[e]), 1)
            wins = _windows(cnt, ())
            win2.append(tuple(wins))
            idxs, av = _pack_cell(e, dcore, dcol, srow, v2e, wins)
            i2.append(idxs)
            v2.append(av)
    C2 = sum(len(w) for w in win2)
    a2i = np.concatenate(i2, axis=0).transpose(1, 2, 0)          # [NC, P, C2]
    a2v = np.stack(v2, axis=0).transpose(1, 2, 0, 3).reshape(
        NCORES, P, len(v2) * 512)

    # --- node features ---
    xtab = x.astype(np.float16)                                  # [N, 512]
    xt = np.zeros((NCORES, P, NG * 2048), np.float16)
    for c in range(NCORES):
        ids = np.arange(c * NLOC, (c + 1) * NLOC)
        arr = np.zeros((NPAD, IN), np.float32)
        arr[pos[ids]] = x[ids]
        # xt[c][p, g*2048 + kc*512 + col] = arr[g*512 + col, kc*128 + p]
        xt[c] = np.ascontiguousarray(
            arr.T.reshape(4, P, NG, 512).transpose(1, 2, 0, 3)
            .reshape(P, NG * 2048)).astype(np.float16)

    # --- weight folding (BN eval folded into layer-2 weights) ---
    f64 = np.float64
    s = np.asarray(gamma, f64) / np.sqrt(np.asarray(run_var, f64) + BN_EPS)
    t = np.asarray(beta, f64) - np.asarray(run_mean, f64) * s

    # wl/wr: [P, k*2048 + kc*512 + j] = Wx[k].T[kc*128+p, j]
    def _pmaj_w(W5):
        wt = np.asarray(W5, np.float32).transpose(0, 2, 1).astype(np.float16)
        return np.ascontiguousarray(
            wt.reshape(NREL, 4, P, HID).transpose(2, 0, 1, 3)
            .reshape(P, NREL * 4 * HID))
    wlt = _pmaj_w(Wl5)
    wrt = _pmaj_w(Wr5)

    Wtab = np.concatenate([np.asarray(Wmu_l, f64), np.asarray(Wlv_l, f64)], 0)
    Wself = np.concatenate([np.asarray(Wmu_r, f64), np.asarray(Wlv_r, f64)], 0)
    Wall = np.concatenate([Wtab * s[None, :], Wself * s[None, :]], 0)  # [1024, 2560]
    # wallt[p, r*1024 + j] = Wall.T[r*128+p, j]
    wallt = np.ascontiguousarray(
        Wall.T.astype(np.float16).reshape(20, P, 1024).transpose(1, 0, 2)
        .reshape(P, 20 * 1024))

    tW = (Wtab @ t).astype(np.float32)                                  # [512]
    bself = (Wself @ t + np.concatenate(
        [np.asarray(bmu, f64), np.asarray(blv, f64)])).astype(np.float32)
    pbias = np.ascontiguousarray(
        np.tile(np.concatenate([tW, bself])[None, :], (P, 1)))  # [128, 1024]
    bsb = np.ascontiguousarray(bself.reshape(4, P).T)           # [128, 4]

    blb = np.ascontiguousarray(
        np.asarray(bl5, np.float32).reshape(NREL * 4, P).T)     # [128, 20]

    meta = (tuple(win1), tuple(win2))
    in_maps = []
    for c in range(NCORES):
        in_maps.append({
            "xtab": xtab, "xt": xt[c],
            "a1i": np.ascontiguousarray(a1i[c]),
            "a1v": np.ascontiguousarray(a1v[c]),
            "a2i": np.ascontiguousarray(a2i[c]),
            "a2v": np.ascontiguousarray(a2v[c]),
            "wlt": wlt, "wrt": wrt, "wallt": wallt,
            "blb": blb, "pbias": pbias, "bsb": bsb,
        })
    return meta, in_maps, pos


# ----------------------------------------------------------------------------
# Device kernel
# ----------------------------------------------------------------------------

def _build(meta):
    import concourse.bacc as bacc
    import concourse.bass as bass
    import concourse.tile as tile
    import concourse.mybir as mybir

    win1, win2 = meta
    nch1 = [len(w) for w in win1]          # 25 cells, (g, k) order
    base1 = np.concatenate([[0], np.cumsum(nch1)[:-1]])
    C1 = int(np.sum(nch1))
    nch2 = [len(w) for w in win2]          # 25 cells, (s, d) order
    base2 = np.concatenate([[0], np.cumsum(nch2)[:-1]])
    C2 = int(np.sum(nch2))

    f16, f32, i32 = mybir.dt.float16, mybir.dt.float32, mybir.dt.int32

    nc = bacc.Bacc("TRN2", target_bir_lowering=False, debug=False,
                   num_devices=NCORES)

    xtab_t = nc.dram_tensor("xtab", [N, IN], f16, kind="ExternalInput")
    xt_t = nc.dram_tensor("xt", [P, NG * 2048], f16, kind="ExternalInput")
    a1i_t = nc.dram_tensor("a1i", [P, C1], i32, kind="ExternalInput")
    a1v_t = nc.dram_tensor("a1v", [P, 25 * 512], f16, kind="ExternalInput")
    a2i_t = nc.dram_tensor("a2i", [P, C2], i32, kind="ExternalInput")
    a2v_t = nc.dram_tensor("a2v", [P, 25 * 512], f16, kind="ExternalInput")
    wlt_t = nc.dram_tensor("wlt", [P, NREL * 2048], f16, kind="ExternalInput")
    wrt_t = nc.dram_tensor("wrt", [P, NREL * 2048], f16, kind="ExternalInput")
    wallt_t = nc.dram_tensor("wallt", [P, 20 * 1024], f16, kind="ExternalInput")
    blb_t = nc.dram_tensor("blb", [P, NREL * 4], f32, kind="ExternalInput")
    pbias_t = nc.dram_tensor("pbias", [P, 1024], f32, kind="ExternalInput")
    bsb_t = nc.dram_tensor("bsb", [P, 4], f32, kind="ExternalInput")
    out_t = nc.dram_tensor("out", [P, NG * 2048], f16, kind="ExternalOutput")

    warm_t = nc.dram_tensor("warm", [P, 512], f16, kind="Internal")
    ag_in = nc.dram_tensor("ag_in", [NPAD, 512], f16, kind="Internal")
    ag_tabs = [nc.dram_tensor(f"ag_tab{s}", [NCORES * 512, 512], f16,
                              kind="Internal", addr_space="Shared")
               for s in range(NG)]

    with tile.TileContext(nc) as tc:
        with (
            tc.tile_pool(name="constp", bufs=1) as constp,
            tc.tile_pool(name="resp", bufs=1) as resp,
            tc.tile_pool(name="iop", bufs=3) as iop,
            tc.tile_pool(name="actp", bufs=2) as actp,
            tc.tile_pool(name="psum", bufs=4, space="PSUM") as pp,
        ):
            # ---- constants / index tiles ----
            blb_sb = constp.tile([P, NREL * 4], f32, name="blb_sb", tag="blb")
            nc.sync.dma_start(out=blb_sb[:], in_=blb_t.ap())
            pbias_sb = constp.tile([P, 1024], f32, name="pbias_sb", tag="pb")
            nc.sync.dma_start(out=pbias_sb[:], in_=pbias_t.ap())
            bsb_sb = constp.tile([P, 4], f32, name="bsb_sb", tag="bsb")
            nc.sync.dma_start(out=bsb_sb[:], in_=bsb_t.ap())

            idx1_sb = resp.tile([P, C1], i32, name="idx1_sb", tag="idx1")
            nc.gpsimd.dma_start(out=idx1_sb[:], in_=a1i_t.ap())
            idx2_sb = resp.tile([P, C2], i32, name="idx2_sb", tag="idx2")
            nc.gpsimd.dma_start(out=idx2_sb[:], in_=a2i_t.ap())

            # ---- resident tensors ----
            wl_sb = resp.tile([P, NREL * 2048], f16, name="wl_sb", tag="wl")
            wr_sb = resp.tile([P, NREL * 2048], f16, name="wr_sb", tag="wr")
            wall_sb = resp.tile([P, 20 * 1024], f16, name="wall_sb", tag="wall")
            sf_sb = resp.tile([P, 20 * 512], f16, name="sf_sb", tag="sf")
            acc_sb = resp.tile([P, NG * 2048], f16, name="acc_sb", tag="acc")

            def fetch_xt(g):
                xtg = iop.tile([P, 2048], f16, name=f"xt_{g}", tag="xt",
                               bufs=3)
                nc.scalar.dma_start(
                    out=xtg[:], in_=xt_t.ap()[:, g * 2048:(g + 1) * 2048])
                return xtg

            # spread the input streams: xt on scalar; wall halves are
            # emitted after the wl/wr streams below
            xtg0 = fetch_xt(0)

            # ---------------- helpers ----------------
            def fetch_av1(g, k):
                cell = g * NREL + k
                av = iop.tile([P, 512], f16, name=f"a1_{g}_{k}", tag="av1",
                              bufs=6)
                nc.sync.dma_start(
                    out=av[:], in_=a1v_t.ap()[:, cell * 512:(cell + 1) * 512])
                return av

            def fetch_av2(s, d):
                cell = s * NG + d
                av = iop.tile([P, 512], f16, name=f"a2_{s}_{d}", tag="av2",
                              bufs=6)
                nc.sync.dma_start(
                    out=av[:], in_=a2v_t.ap()[:, cell * 512:(cell + 1) * 512])
                return av

            # group-0 one-hot values first on sync, then the weights
            # (wl on sync, wr on scalar, so the streams race the dense MMs)
            avs1 = [fetch_av1(0, k) for k in range(NREL)]
            for k in range(NREL):
                nc.sync.dma_start(
                    out=wl_sb[:, k * 2048:(k + 1) * 2048],
                    in_=wlt_t.ap()[:, k * 2048:(k + 1) * 2048])
                nc.scalar.dma_start(
                    out=wr_sb[:, k * 2048:(k + 1) * 2048],
                    in_=wrt_t.ap()[:, k * 2048:(k + 1) * 2048])
            for r in range(20):
                nc.sync.dma_start(
                    out=wall_sb[:, r * 1024:r * 1024 + 512],
                    in_=wallt_t.ap()[:, r * 1024:r * 1024 + 512])
            for r in range(20):
                nc.sync.dma_start(
                    out=wall_sb[:, r * 1024 + 512:(r + 1) * 1024],
                    in_=wallt_t.ap()[:, r * 1024 + 512:(r + 1) * 1024])

            # ---- PE warm-up while the first gathers stream in ----
            wu = constp.tile([P, 512], f16, name="wu", tag="wu")
            nc.vector.memset(wu[:], 0.0)
            wu_ps = pp.tile([P, 512], f32, space="PSUM", name="wu_ps",
                            tag="sm", bufs=4)
            for i in range(16):
                nc.tensor.matmul(out=wu_ps[:], lhsT=wu[:, 0:P], rhs=wu[:],
                                 start=(i == 0), stop=(i == 15))
            nc.vector.tensor_copy(out=wu[:], in_=wu_ps[:])
            nc.scalar.dma_start(out=warm_t.ap(), in_=wu[:])
            def l1_gathers(g, k):
                cell = g * NREL + k
                cbase = int(base1[cell])
                gts = []
                for ci in range(nch1[cell]):
                    gth = iop.tile([P, 512], f16, name=f"g1_{g}_{k}_{ci}",
                                   tag="gth", bufs=24)
                    nc.gpsimd.indirect_dma_start(
                        out=gth[:], out_offset=None,
                        in_=xtab_t.ap(),
                        in_offset=bass.IndirectOffsetOnAxis(
                            ap=idx1_sb[:, cbase + ci:cbase + ci + 1], axis=0))
                    gts.append(gth)
                return gts

            def l2_gathers(s, d):
                cell = s * NG + d
                cbase = int(base2[cell])
                gts = []
                for ci in range(nch2[cell]):
                    gth = iop.tile([P, 512], f16, name=f"g2_{s}_{d}_{ci}",
                                   tag="gth", bufs=24)
                    nc.gpsimd.indirect_dma_start(
                        out=gth[:], out_offset=None,
                        in_=ag_tabs[s].ap(),
                        in_offset=bass.IndirectOffsetOnAxis(
                            ap=idx2_sb[:, cbase + ci:cbase + ci + 1], axis=0))
                    gts.append(gth)
                return gts

            def l1_agg(g, k, gts, av):
                """aggregation for cell (g, k) -> mean tiles (Act copies)"""
                cell = g * NREL + k
                aggs = [pp.tile([P, 1024], f32, space="PSUM",
                                name=f"agg_{g}_{k}_{h}", tag="big", bufs=2)
                        for h in range(2)]
                for cc in range(4):
                    tgt = aggs[cc // 2]
                    off = (cc % 2) * 512
                    for ci, (lo, hi) in enumerate(win1[cell]):
                        nc.tensor.matmul(
                            out=tgt[:, off + lo:off + hi],
                            lhsT=gts[ci][:, cc * P:(cc + 1) * P],
                            rhs=av[:, lo:hi], start=True, stop=True)
                means = []
                for kc in range(4):
                    m = actp.tile([P, 512], f16, name=f"mean_{g}_{k}_{kc}",
                                  tag=f"mean{kc}")
                    nc.scalar.copy(
                        out=m[:],
                        in_=aggs[kc // 2][:, (kc % 2) * 512:
                                          (kc % 2) * 512 + 512])
                    means.append(m)
                return means

            def l1_dense(g, k, means, xtg):
                """dense h = relu(Wl@mean + Wr@x + b) for cell (g, k)"""
                rt = actp.tile([P, 2048], f16, name=f"rt_{g}_{k}",
                               tag=f"rt{k}", bufs=1)
                for mc in range(4):
                    h_ps = pp.tile([P, 512], f32, space="PSUM",
                                   name=f"h_{g}_{k}_{mc}", tag="sm", bufs=4)
                    # self side first: it does not depend on the mean
                    # copies, which land on Act while these MMs run
                    for kc in range(4):
                        nc.tensor.matmul(
                            out=h_ps[:],
                            lhsT=wr_sb[:, k * 2048 + kc * 512 + mc * P:
                                       k * 2048 + kc * 512 + (mc + 1) * P],
                            rhs=xtg[:, kc * 512:(kc + 1) * 512],
                            start=(kc == 0), stop=False)
                    for kc in range(4):
                        nc.tensor.matmul(
                            out=h_ps[:],
                            lhsT=wl_sb[:, k * 2048 + kc * 512 + mc * P:
                                       k * 2048 + kc * 512 + (mc + 1) * P],
                            rhs=means[kc][:], start=False, stop=(kc == 3))
                    nc.vector.tensor_scalar(
                        out=rt[:, mc * 512:(mc + 1) * 512], in0=h_ps[:],
                        scalar1=blb_sb[:, k * 4 + mc:k * 4 + mc + 1],
                        scalar2=0.0,
                        op0=mybir.AluOpType.add, op1=mybir.AluOpType.max)
                return rt

            def l2_cell(s, d, gts, av):
                """one (src-group, dst-group) partial-sum pass into acc.
                Feature-major: out[tab-chunk cc, node cols]."""
                cell = s * NG + d
                pss = [pp.tile([P, 1024], f32, space="PSUM",
                               name=f"m2_{s}_{d}_{h}", tag="big", bufs=2)
                       for h in range(2)]
                for cc in range(4):
                    tgt = pss[cc // 2]
                    off = (cc % 2) * 512
                    for ci, (lo, hi) in enumerate(win2[cell]):
                        nc.tensor.matmul(
                            out=tgt[:, off + lo:off + hi],
                            lhsT=gts[ci][:, cc * P:(cc + 1) * P],
                            rhs=av[:, lo:hi], start=True, stop=True)
                for h in range(2):
                    a = acc_sb[:, d * 2048 + h * 1024:d * 2048 + h * 1024 + 1024]
                    if s == 0:
                        nc.vector.tensor_copy(out=a, in_=pss[h][:])
                    else:
                        nc.vector.tensor_tensor(out=a, in0=pss[h][:], in1=a,
                                                op=mybir.AluOpType.add)

            def proj_tab(g, rts):
                """node-major tab projection for group g -> ag_in rows"""
                for ncx in range(4):
                    ps = pp.tile([P, 512], f32, space="PSUM",
                                 name=f"pt_{g}_{ncx}", tag="sm", bufs=4)
                    for r in range(20):
                        nc.tensor.matmul(
                            out=ps[:],
                            lhsT=rts[r // 4][:, (r % 4) * 512 + ncx * P:
                                             (r % 4) * 512 + (ncx + 1) * P],
                            rhs=wall_sb[:, r * 1024:r * 1024 + 512],
                            start=(r == 0), stop=(r == 19))
                    agin = actp.tile([P, 512], f16,
                                     name=f"agin_{g}_{ncx}", tag="agin", bufs=1)
                    nc.vector.tensor_tensor(
                        out=agin[:], in0=ps[:], in1=pbias_sb[:, 0:512],
                        op=mybir.AluOpType.add)
                    nc.sync.dma_start(
                        out=ag_in.ap()[g * 512 + ncx * P:
                                       g * 512 + (ncx + 1) * P, :],
                        in_=agin[:])

            def proj_self(g, rts):
                """feature-major self projection for group g -> sf slices"""
                for sc in range(4):
                    ps = pp.tile([P, 512], f32, space="PSUM",
                                 name=f"psf_{g}_{sc}", tag="sm", bufs=4)
                    for r in range(20):
                        nc.tensor.matmul(
                            out=ps[:],
                            lhsT=wall_sb[:, r * 1024 + 512 + sc * P:
                                         r * 1024 + 512 + (sc + 1) * P],
                            rhs=rts[r // 4][:, (r % 4) * 512:
                                            (r % 4 + 1) * 512],
                            start=(r == 0), stop=(r == 19))
                    nc.vector.tensor_scalar(
                        out=sf_sb[:, (g * 4 + sc) * 512:
                                  (g * 4 + sc + 1) * 512],
                        in0=ps[:], scalar1=bsb_sb[:, sc:sc + 1],
                        scalar2=None, op0=mybir.AluOpType.add)

            def emit_ag(g):
                nc.gpsimd.collective_compute(
                    "AllGather", mybir.AluOpType.bypass,
                    replica_groups=[list(range(NCORES))],
                    ins=[ag_in.ap()[g * 512:(g + 1) * 512, :]],
                    outs=[ag_tabs[g].ap()])

            def combine(d):
                for cc in range(4):
                    ob = actp.tile([P, 512], f16, name=f"ob_{d}_{cc}",
                                   tag="ob", bufs=4)
                    nc.vector.tensor_tensor(
                        out=ob[:],
                        in0=acc_sb[:, d * 2048 + cc * 512:
                                   d * 2048 + (cc + 1) * 512],
                        in1=sf_sb[:, (d * 4 + cc) * 512:
                                  (d * 4 + cc + 1) * 512],
                        op=mybir.AluOpType.add)
                    q = [nc.sync, nc.scalar, nc.gpsimd][(d * 4 + cc) % 3]
                    q.dma_start(
                        out=out_t.ap()[:, d * 2048 + cc * 512:
                                       d * 2048 + (cc + 1) * 512],
                        in_=ob[:])

            # ---------------- main schedule ----------------
            all_rts = {}
            xtg = xtg0
            for g in range(NG):
                if g > 0:
                    avs1 = [fetch_av1(g, k) for k in range(NREL)]
                    xtg = fetch_xt(g)
                avs2 = [fetch_av2(g - 2, d) for d in range(NG)] \
                    if g >= 2 else None
                rts = []
                for k in range(NREL):
                    # gathers first (gpsimd queue), then compute
                    gts1 = l1_gathers(g, k)
                    gts2 = l2_gathers(g - 2, k) if g >= 2 else None
                    # AllGather of the previous group, emitted mid-stream
                    # once its input is surely written
                    if k == 2 and g >= 1:
                        emit_ag(g - 1)
                    means = l1_agg(g, k, gts1, avs1[k])
                    rts.append(l1_dense(g, k, means, xtg))
                    if gts2 is not None:
                        l2_cell(g - 2, k, gts2, avs2[k])
                # group 4's self projection is deferred into the AG_4 window
                proj_tab(g, rts)
                if g < NG - 1:
                    proj_self(g, rts)
                else:
                    all_rts[g] = rts
            # tail: AG_4 fires as soon as its agin lands; the deferred
            # self projection and s=3 pass hide its wire time
            emit_ag(NG - 1)
            avt2 = [fetch_av2(3, d) for d in range(NG)]
            tail2 = [l2_gathers(3, d) for d in range(NG)]
            avt3 = [fetch_av2(4, d) for d in range(NG)]
            tail3 = [l2_gathers(4, d) for d in range(NG)]
            proj_self(4, all_rts[4])
            for d in range(NG):
                l2_cell(3, d, tail2[d], avt2[d])
            for d in range(NG):
                l2_cell(4, d, tail3[d], avt3[d])
                combine(d)

    nc.compile()
    return nc


# ----------------------------------------------------------------------------
# Entry point
# ----------------------------------------------------------------------------

_CACHE = {}


def build_and_run(inputs, trace=False, trace_kwargs=None):
    from concourse import bass_utils

    meta, in_maps, pos = _preprocess(**inputs)
    if meta not in _CACHE:
        _CACHE[meta] = _build(meta)
    nc = _CACHE[meta]
    res = bass_utils.run_bass_kernel_spmd(
        nc, in_maps, core_ids=list(range(NCORES)),
        trace=trace, **(trace_kwargs or {}))

    mu = np.empty((N, OUT), np.float32)
    lv = np.empty((N, OUT), np.float32)
    for c in range(NCORES):
        raw = np.asarray(res.results[c]["out"], np.float32)
        blk = raw.reshape(P, NG, 4, 512).transpose(2, 0, 1, 3).reshape(
            512, NPAD)                          # [512 ch, 2560 positions]
        ids = np.arange(c * NLOC, (c + 1) * NLOC)
        mu[ids] = blk[0:OUT, pos[ids]].T
        lv[ids] = blk[OUT:2 * OUT, pos[ids]].T
    return (mu, lv), res


def kernel(**inputs):
    out, _ = build_and_run(inputs, trace=False)
    return out
